# revision 1
# baseline (speedup 1.0000x reference)
"""Trainium2 Bass kernel for gated-attention pooling (B=8, N=8192, L=1024, D=256).

Reference computation (per batch b):
    a = tanh(x Wa + ba); g = sigmoid(x Wb + bb)
    A = (a*g) Wc + bc                      # [N] scores
    w = softmax(mask(A, lengths))          # over N
    out[b] = w @ x[b]                      # [L]

Strategy:
  - softmax decomposes into unnormalized sums  P = sum_n exp(A_n) x_n  and
    S = sum_n exp(A_n), so work shards freely across the 8 cores at row
    granularity; all batch grouping plus the final P/S division happen on the
    host.  Only rows n < lengths[b] matter (masked rows get weight 0), so the
    host packs just the valid rows, padded to 512-row super-tiles (STs), and
    spreads them evenly over the cores.  One SPMD program processes S STs per
    core; padding rows carry a 0 multiplicative mask on exp(A).
  - The host uploads x pre-transposed (L-major) so the PE can contract over L
    directly (the PE contracts over the partition dim).  Scores use fp32
    operands tagged float32r: full PE rate at N=512 with ~tf32 precision.
    sigmoid(z) = 0.5*tanh(z/2) + 0.5 keeps ACT inside one table set
    (exp_and_others has exp+tanh but not sigmoid); the 0.5 factors are folded
    into Wc and the tanh input scale on the host.
  - Per ST on device:
      scores:  4 weight groups (Wa/Wb x 2 D-chunks) x 8 L-chunks of
               [128,128]x[128,512] float32r matmuls accumulated in PSUM
      ACT:     tanh (per-partition bias via the activation bias port) -> a, t
      DVE:     m = (t+1)*a   (one scalar_tensor_tensor per D-chunk)
      A row:   [1,512] = two Wc-chunk matmuls (M=1)
      u row:   ACT exp(A + bc) -> bf16, then DVE multiply by the 0/1 mask row
      pooling: PE broadcasts u across partitions (K=1 matmul with a ones
               column); DVE scalar_tensor_tensor computes the per-L-chunk
               column sums  sum_n u_n * xT[l, n]  via accum_out, one output
               column per (ST, L-chunk).
  - Outputs per core: pp [128, S*8] (per-ST per-L-chunk partial pools) and
    urow [1, S*512] (unnormalized weights); the host reduces them per batch
    and divides.

MODE:
  "f32r" (default) - fp32 x on the wire (float32r matmuls), most accurate,
                     PE-bound at the same rate as bf16.
  "bf16"           - bf16 x single upload, half the DMA bytes.
  "fp8"            - e4m3 DoubleRow score matmuls (~1.8x faster overall),
                     plus a bf16 x upload for the pooling; ~3x the error.
"""

import numpy as np
import ml_dtypes

BF16 = ml_dtypes.bfloat16
FP8 = ml_dtypes.float8_e4m3
B, N, L, D = 8, 8192, 1024, 256
ST = 512          # rows per super-tile
LC = L // 128     # 8 L-chunks

MODE = "f32r"     # "f32r" | "bf16" | "fp8"
W_SCALE = 64.0    # fp8 weight pre-scale, undone by the tanh input scale
XBUFS = 5         # x-tile buffering (deep prefetch; ~191KB/partition SBUF total)
WBUFS = 5         # working-tile buffering (act/m/u/scratch)

_cache = {}


def _split_multiwait(nc, max_waits=1):
    """This container's walrus rejects instructions carrying more than a
    couple of semaphore waits ("Too many sync wait commands").  Split extras
    into same-engine single-wait NoOps placed immediately before."""
    import concourse.mybir as mybir

    for f in nc.m.functions:
        for bb in f.blocks:
            insts = bb.instructions  # live list
            new = []
            for inst in insts:
                si = inst.sync_info
                if si is not None and len(si.on_wait) > max_waits:
                    waits = list(si.on_wait)
                    for k, w in enumerate(waits[:-max_waits]):
                        nop = mybir.InstNoOp(
                            name=f"{inst.name}-wsplit{k}", ins=[], outs=[]
                        )
                        nop.engine = inst.engine
                        nop.debug = inst.debug
                        nop.sync_info = mybir.SyncInfo(on_wait=[w], on_update=[])
                        new.append(nop)
                    inst.sync_info = mybir.SyncInfo(
                        on_wait=waits[-max_waits:], on_update=list(si.on_update)
                    )
                new.append(inst)
            insts[:] = new


def _build(S, reps=1, mode=None):
    if mode is None:
        mode = MODE
    import concourse.bass as bass
    import concourse.mybir as mybir
    import concourse.tile as tile
    from contextlib import ExitStack

    f32 = mybir.dt.float32
    f32r = mybir.dt.float32r
    bf16 = mybir.dt.bfloat16
    fp8e4 = mybir.dt.float8e4
    ACT = mybir.ActivationFunctionType
    ALU = mybir.AluOpType

    xdt = {"f32r": f32r, "bf16": bf16, "fp8": bf16}[mode]
    wdt = {"f32r": f32r, "bf16": bf16, "fp8": fp8e4}[mode]

    nc = bass.Bass()
    xt_d = nc.declare_dram_parameter("xt", [S, 128, LC, ST], xdt, isOutput=False)
    if mode == "fp8":
        xt8_d = nc.declare_dram_parameter(
            "xt8", [S, 128, LC, ST], fp8e4, isOutput=False
        )
    mrow_d = nc.declare_dram_parameter("mrow", [1, S * ST], f32, isOutput=False)
    wsb_d = nc.declare_dram_parameter("wsb", [128, 32 * 128], wdt, isOutput=False)
    bcol_d = nc.declare_dram_parameter("bcol", [128, 4], f32, isOutput=False)
    wcc_d = nc.declare_dram_parameter("wcc", [128, 2], bf16, isOutput=False)
    bcs_d = nc.declare_dram_parameter("bcs", [1, 1], f32, isOutput=False)
    ones_d = nc.declare_dram_parameter("ones", [1, 128], f32r, isOutput=False)
    pp_d = nc.declare_dram_parameter("pp", [128, S * LC], f32, isOutput=True)
    urow_d = nc.declare_dram_parameter("urow", [1, S * ST], f32r, isOutput=True)

    with tile.TileContext(nc) as tc, ExitStack() as ctx:
        const = ctx.enter_context(tc.tile_pool(name="const", bufs=1))
        outp = ctx.enter_context(tc.tile_pool(name="outp", bufs=1))
        xpool = ctx.enter_context(tc.tile_pool(name="x", bufs=XBUFS))
        apool = ctx.enter_context(tc.tile_pool(name="act", bufs=WBUFS))
        mpool = ctx.enter_context(tc.tile_pool(name="m", bufs=WBUFS))
        upool = ctx.enter_context(tc.tile_pool(name="u", bufs=WBUFS))
        spool = ctx.enter_context(tc.tile_pool(name="scr", bufs=WBUFS))
        scp = ctx.enter_context(tc.tile_pool(name="scp", bufs=4, space="PSUM"))
        ap_ps = ctx.enter_context(tc.tile_pool(name="apps", bufs=2, space="PSUM"))
        bc_ps = ctx.enter_context(tc.tile_pool(name="bcps", bufs=2, space="PSUM"))

        w_sb = const.tile([128, 32 * 128], wdt, tag="w")
        nc.sync.dma_start(w_sb[:], wsb_d[:])
        bcol = const.tile([128, 4], f32, tag="bcol")
        nc.sync.dma_start(bcol[:], bcol_d[:])
        wcc = const.tile([128, 2], bf16, tag="wcc")
        nc.sync.dma_start(wcc[:], wcc_d[:])
        bcs = const.tile([1, 1], f32, tag="bcs")
        nc.sync.dma_start(bcs[:], bcs_d[:])
        ones = const.tile([1, 128], f32r, tag="ones")
        nc.sync.dma_start(ones[:], ones_d[:])
        msb = const.tile([1, S * ST], f32, tag="mrow")
        nc.sync.dma_start(msb[:], mrow_d[:])

        pp_sb = outp.tile([128, S * LC], f32, tag="pp")
        u_row = outp.tile([1, S * ST], f32r, tag="urow")

        def scores_part(st):
                xts = xpool.tile([128, LC * ST], xdt, tag="xt")
                nc.sync.dma_start(xts[:], xt_d[st].rearrange("p c n -> p (c n)"))
                if mode == "fp8":
                    xts8 = xpool.tile([128, LC * ST], fp8e4, tag="xt8")
                    nc.sync.dma_start(
                        xts8[:], xt8_d[st].rearrange("p c n -> p (c n)")
                    )
                    xts8_3d = xts8[:].rearrange("p (c n) -> p c n", c=LC)
                    w_3d = w_sb[:].rearrange("p (k m) -> p k m", k=32)
                # ---- scores: 4 weight groups (a0, a1, t0, t1) ----
                acts = []
                for wg in range(4):
                    ps = scp.tile([128, ST], f32, tag="ps")
                    if mode == "fp8":
                        for l2 in range(LC // 2):
                            nc.tensor.matmul(
                                ps[:],
                                w_3d[:, wg * LC + 2 * l2 : wg * LC + 2 * l2 + 2, :],
                                xts8_3d[:, 2 * l2 : 2 * l2 + 2, :],
                                start=(l2 == 0),
                                stop=(l2 == LC // 2 - 1),
                                perf_mode=mybir.MatmulPerfMode.DoubleRow,
                            )
                    else:
                        for lc in range(LC):
                            nc.tensor.matmul(
                                ps[:],
                                w_sb[
                                    :, (wg * LC + lc) * 128 : (wg * LC + lc + 1) * 128
                                ],
                                xts[:, lc * ST : (lc + 1) * ST],
                                start=(lc == 0),
                                stop=(lc == LC - 1),
                            )
                    dst = apool.tile([128, ST], bf16, tag=f"act{wg}")
                    sc = (1.0 if wg < 2 else 0.5) / (W_SCALE if mode == "fp8" else 1.0)
                    nc.scalar.activation(
                        dst[:], ps[:], ACT.Tanh, bias=bcol[:, wg : wg + 1], scale=sc
                    )
                    acts.append(dst)
                return xts, acts

        def tail_part(st, xts, acts):
                a0, a1, t0, t1 = acts
                # ---- m = (t + 1) * a  (per D-chunk) ----
                m0 = mpool.tile([128, ST], bf16, tag="m0")
                nc.vector.scalar_tensor_tensor(
                    m0[:], t0[:], 1.0, a0[:], ALU.add, ALU.mult
                )
                m1 = mpool.tile([128, ST], bf16, tag="m1")
                nc.vector.scalar_tensor_tensor(
                    m1[:], t1[:], 1.0, a1[:], ALU.add, ALU.mult
                )
                # ---- A row = m @ (Wc/2) ----
                aps = ap_ps.tile([1, ST], f32, tag="A")
                nc.tensor.matmul(aps[:], wcc[:, 0:1], m0[:], start=True, stop=False)
                nc.tensor.matmul(aps[:], wcc[:, 1:2], m1[:], start=False, stop=True)
                # ---- u = exp(A + bc) * mask01 ----
                useg = u_row[0:1, st * ST : (st + 1) * ST]
                nc.scalar.activation(useg, aps[:], ACT.Exp, bias=bcs[0:1, 0:1])
                nc.vector.tensor_tensor(
                    useg, useg, msb[0:1, st * ST : (st + 1) * ST], ALU.mult
                )
                # ---- broadcast u across partitions (K=1 f32r matmul) ----
                bps = bc_ps.tile([128, ST], f32, tag="bc")
                nc.tensor.matmul(bps[:], ones[:], useg, start=True, stop=True)
                ubc = upool.tile([128, ST], f32, tag="ubc")
                nc.scalar.copy(ubc[:], bps[:])
                # ---- pooling: per L-chunk column sums into pp ----
                xsrc = xts[:].bitcast(f32) if mode == "f32r" else xts[:]
                for c2 in range(LC):
                    to = spool.tile([128, ST], bf16, tag="ttr")
                    nc.vector.scalar_tensor_tensor(
                        to[:],
                        xsrc[:, c2 * ST : (c2 + 1) * ST],
                        1.0,
                        ubc[:],
                        ALU.bypass,
                        ALU.mult,
                        accum_out=pp_sb[:, st * LC + c2 : st * LC + c2 + 1],
                    )

        # sequential emission schedules best (a software-pipelined variant —
        # ST s scores emitted before ST s-1 tail — measured ~10us slower)
        for _rep in range(reps):
            for st in range(S):
                tail_part(st, *scores_part(st))

        nc.sync.dma_start(pp_d[:], pp_sb[:])
        nc.sync.dma_start(urow_d[:], u_row[:])

    _split_multiwait(nc)
    return nc


def _get_program(S, reps=1):
    key = (S, reps, MODE)
    if key not in _cache:
        _cache[key] = _build(S, reps)
    return _cache[key]


def _prepare(x, lengths, Wa, ba, Wb, bb, Wc, bc):
    """Host-side packing.  Returns (S, in_maps, slot_map) where slot_map[c][k]
    is (batch, row0, n_valid) or None for padding slots."""
    x = np.asarray(x, dtype=np.float32)
    lengths = np.asarray(lengths, dtype=np.int64)
    Wa = np.asarray(Wa, dtype=np.float32)
    ba = np.asarray(ba, dtype=np.float32)
    Wb = np.asarray(Wb, dtype=np.float32)
    bb = np.asarray(bb, dtype=np.float32)
    Wc = np.asarray(Wc, dtype=np.float32).reshape(D)
    bc = float(np.asarray(bc, dtype=np.float32).reshape(()))

    slots = []
    for b in range(B):
        ln = int(max(1, min(N, lengths[b])))
        for k in range((ln + ST - 1) // ST):
            slots.append((b, k * ST, min(ST, ln - k * ST)))
    n_cores = 8
    S = max(1, (len(slots) + n_cores - 1) // n_cores)

    # weights, shared by all cores
    xdt = {"f32r": np.float32, "bf16": BF16, "fp8": BF16}[MODE]
    wdt = {"f32r": np.float32, "bf16": BF16, "fp8": FP8}[MODE]
    wmul = W_SCALE if MODE == "fp8" else 1.0
    wsb = np.zeros((128, 32, 128), dtype=wdt)
    for g, W in enumerate((Wa, Wb)):
        for dc in range(2):
            for lc in range(LC):
                # chunk [K=128 (L), M=128 (D)]
                wsb[:, (g * 2 + dc) * LC + lc, :] = (
                    W[lc * 128 : (lc + 1) * 128, dc * 128 : (dc + 1) * 128] * wmul
                ).astype(wdt)
    wsb = wsb.reshape(128, 32 * 128)
    bcol = np.stack(
        [ba[0:128], ba[128:256], bb[0:128] * 0.5, bb[128:256] * 0.5], axis=1
    ).astype(np.float32)
    wcc = np.stack([Wc[0:128] * 0.5, Wc[128:256] * 0.5], axis=1).astype(BF16)

    in_maps = []
    slot_map = []
    xcast = x if xdt == np.float32 else x.astype(xdt)
    for c in range(n_cores):
        xt = np.zeros((S, 128, LC, ST), dtype=xdt)
        mrow = np.zeros((S, ST), dtype=np.float32)
        smap = []
        for k in range(S):
            g = c * S + k
            if g < len(slots):
                b, r0, nvalid = slots[g]
                slab = xcast[b, r0 : r0 + nvalid, :]  # [nvalid, 1024]
                # xt[k, p, lc, n] = x[b, r0+n, 128*lc+p]
                xt[k, :, :, :nvalid] = slab.reshape(nvalid, LC, 128).transpose(2, 1, 0)
                mrow[k, :nvalid] = 1.0
                smap.append((b, r0, nvalid))
            else:
                smap.append(None)
        im = {
            "xt": xt,
            "mrow": mrow.reshape(1, S * ST),
            "wsb": wsb,
            "bcol": bcol,
            "wcc": wcc,
            "bcs": np.array([[bc]], dtype=np.float32),
            "ones": np.ones((1, 128), dtype=np.float32),
        }
        if MODE == "fp8":
            im["xt8"] = xt.astype(FP8)
        in_maps.append(im)
        slot_map.append(smap)
    return S, in_maps, slot_map


def _assemble(results, S, slot_map):
    P = np.zeros((B, L), dtype=np.float64)
    Ssum = np.zeros(B, dtype=np.float64)
    for c in range(8):
        pp = np.asarray(results[c]["pp"], dtype=np.float64)  # [128, S*LC]
        ur = np.asarray(results[c]["urow"]).astype(np.float64)[0]  # [S*ST]
        for k, slot in enumerate(slot_map[c]):
            if slot is None:
                continue
            b, r0, nvalid = slot
            cols = pp[:, k * LC : (k + 1) * LC]  # [128, LC]
            P[b] += cols.T.reshape(L)
            Ssum[b] += ur[k * ST : k * ST + nvalid].sum()
    return (P / Ssum[:, None]).astype(np.float32)


def kernel(x, lengths, Wa, ba, Wb, bb, Wc, bc, _reps=1):
    from concourse.bass_utils import run_bass_kernel_spmd

    S, in_maps, slot_map = _prepare(x, lengths, Wa, ba, Wb, bb, Wc, bc)
    nc = _get_program(S, _reps)
    res = run_bass_kernel_spmd(nc, in_maps, list(range(8)))
    return _assemble(res.results, S, slot_map)



# revision 2
# speedup vs baseline: 1.1297x; 1.1297x over previous
"""Trainium2 Bass kernel v2 for gated-attention pooling (B=8, N=8192, L=1024, D=256).

Design (v2, replaces the f32r/DVE-pooling v1):
  - fp8(e4m3)-only x on the wire, TWO layouts per super-tile (ST = 512 rows):
      xt8 [128(l%128), LC=8(l//128), 512(n)]  - l-major, scores matmul rhs
      xn8 [128(n%128), 4(n//128), 1024(l)]    - n-major, pooling matmul lhsT
  - scores: 16 DoubleRow fp8 matmuls per ST; super-tiles processed in groups
    of G so each DR weight pair streams G rhs tiles back-to-back (LDWEIGHTS
    amortized / elided for repeated weights).
  - A scores produced as COLUMNS [128, 4] via 8 tiny matmuls with m as the
    stationary operand (lhsT = m-slice [128(d),128(n-sub)], rhs = Wc/2 col).
    exp() then costs FD=4 on ACT, and u comes out column-shaped.
  - pooling on the PE: per (l-chunk c, n-sub j) a [128,128] fp8 xn8 block is
    loaded as weights and the u column streams through (N=1 matmul),
    accumulating sum_n u_n x[n, l] in PSUM [128, 8].
  - no mask: padded rows have x=0 so their pooling contribution is 0; the
    host sums only the valid prefix of u for the softmax denominator.
  - outputs: pp [128, S*8] f32 (per-(ST,l-chunk) pools), ucols [128, S*4]
    bf16 (u as columns; host masks+sums).
"""

import numpy as np
import ml_dtypes

FP8 = ml_dtypes.float8_e4m3
BF16 = ml_dtypes.bfloat16
B, N, L, D = 8, 8192, 1024, 256
ST = 512
LC = L // 128      # 8 l-chunks
NS = ST // 128     # 4 n-subs
W_SCALE = 64.0     # fp8 weight pre-scale (uniform(-1/32,1/32) -> +-2)

G = 6              # super-tiles per weight-reuse group
XBUFS = 7          # xt8/xn8 tile buffering
WBUFS = 5

_cache = {}


def _split_multiwait(nc, max_waits=1):
    """Walrus in this container rejects >1 semaphore wait per instruction."""
    import concourse.mybir as mybir

    for f in nc.m.functions:
        for bb in f.blocks:
            insts = bb.instructions
            new = []
            for inst in insts:
                si = inst.sync_info
                if si is not None and len(si.on_wait) > max_waits:
                    waits = list(si.on_wait)
                    for k, w in enumerate(waits[:-max_waits]):
                        nop = mybir.InstNoOp(
                            name=f"{inst.name}-wsplit{k}", ins=[], outs=[]
                        )
                        nop.engine = inst.engine
                        nop.debug = inst.debug
                        nop.sync_info = mybir.SyncInfo(on_wait=[w], on_update=[])
                        new.append(nop)
                    inst.sync_info = mybir.SyncInfo(
                        on_wait=waits[-max_waits:], on_update=list(si.on_update)
                    )
                new.append(inst)
            insts[:] = new


def _build(S, reps=1, g_size=None, parts="all", xbufs=None, wbufs=None):
    if g_size is None:
        g_size = G
    if xbufs is None:
        xbufs = max(XBUFS, min(g_size * 2 + 1, 13))
    if wbufs is None:
        wbufs = max(WBUFS, min(g_size + 1, 13))
    import concourse.bass as bass
    import concourse.mybir as mybir
    import concourse.tile as tile
    from contextlib import ExitStack

    f32 = mybir.dt.float32
    bf16 = mybir.dt.bfloat16
    fp8e4 = mybir.dt.float8e4
    ACT = mybir.ActivationFunctionType
    ALU = mybir.AluOpType
    PM = mybir.MatmulPerfMode

    nc = bass.Bass()
    xt8_d = nc.declare_dram_parameter("xt8", [S, 128, LC * ST], fp8e4, isOutput=False)
    xn8_d = nc.declare_dram_parameter("xn8", [S, 128, NS * L], fp8e4, isOutput=False)
    wsb_d = nc.declare_dram_parameter("wsb", [128, 32 * 128], fp8e4, isOutput=False)
    bcol_d = nc.declare_dram_parameter("bcol", [128, 4], f32, isOutput=False)
    wc2_d = nc.declare_dram_parameter("wc2", [128, 2], bf16, isOutput=False)
    bcc_d = nc.declare_dram_parameter("bcc", [128, 1], f32, isOutput=False)
    pp_d = nc.declare_dram_parameter("pp", [128, S * LC], f32, isOutput=True)
    uc_d = nc.declare_dram_parameter("uc", [128, S * NS], bf16, isOutput=True)

    with tile.TileContext(nc) as tc, ExitStack() as ctx:
        const = ctx.enter_context(tc.tile_pool(name="const", bufs=1))
        outp = ctx.enter_context(tc.tile_pool(name="outp", bufs=1))
        xtp = ctx.enter_context(tc.tile_pool(name="xt", bufs=xbufs))
        xnp = ctx.enter_context(tc.tile_pool(name="xn", bufs=xbufs))
        atp = ctx.enter_context(tc.tile_pool(name="at", bufs=wbufs))
        mp = ctx.enter_context(tc.tile_pool(name="m", bufs=wbufs))
        scp = ctx.enter_context(tc.tile_pool(name="scp", bufs=6, space="PSUM"))
        app = ctx.enter_context(tc.tile_pool(name="app", bufs=2, space="PSUM"))

        w_sb = const.tile([128, 32 * 128], fp8e4, tag="w")
        nc.sync.dma_start(w_sb[:], wsb_d[:])
        w3 = w_sb[:].rearrange("p (k m) -> p k m", k=32)
        bcol = const.tile([128, 4], f32, tag="bcol")
        nc.sync.dma_start(bcol[:], bcol_d[:])
        wc2 = const.tile([128, 2], bf16, tag="wc2")
        nc.sync.dma_start(wc2[:], wc2_d[:])
        bcc = const.tile([128, 1], f32, tag="bcc")
        nc.sync.dma_start(bcc[:], bcc_d[:])

        pp_sb = outp.tile([128, S * LC], f32, tag="pp")
        uc_sb = outp.tile([128, S * NS], bf16, tag="uc")

        def load_group(grp):
            xts, xns = {}, {}
            for st in grp:
                xts[st] = xtp.tile([128, LC * ST], fp8e4, tag="xt8", name=f"xt8_{st}")
                nc.sync.dma_start(xts[st][:], xt8_d[st])
                xns[st] = xnp.tile([128, NS * L], fp8e4, tag="xn8", name=f"xn8_{st}")
                nc.sync.dma_start(xns[st][:], xn8_d[st])
            return xts, xns

        def scores(grp, xts):
            """DR score matmuls + tanh + gates for each st in grp.
            Returns {st: m_tile}."""
            ps = {}
            for wg in range(4):
                for st in grp:
                    ps[(st, wg)] = scp.tile([128, ST], f32, tag="ps", name=f"ps_{st}_{wg}")
            for wg in range(4):
                for pair in range(4):
                    w = w3[:, wg * 8 + 2 * pair : wg * 8 + 2 * pair + 2, :]
                    for st in grp:
                        x3 = xts[st][:].rearrange("p (c n) -> p c n", c=LC)
                        nc.tensor.matmul(
                            ps[(st, wg)][:],
                            w,
                            x3[:, 2 * pair : 2 * pair + 2, :],
                            start=(pair == 0),
                            stop=(pair == 3),
                            perf_mode=PM.DoubleRow,
                        )
                # tanh for this weight group as soon as its banks fill
                for st in grp:
                    if wg == 0:
                        self_at = atp.tile([128, 4 * ST], bf16, tag="at")
                        ats[st] = self_at
                    nc.scalar.activation(
                        ats[st][:, wg * ST : (wg + 1) * ST],
                        ps[(st, wg)][:],
                        ACT.Tanh,
                        bias=bcol[:, wg : wg + 1],
                        scale=(1.0 if wg < 2 else 0.5) / W_SCALE,
                    )
            ms = {}
            for st in grp:
                at = ats.pop(st)
                m = mp.tile([128, 2 * ST], fp8e4, tag="m")
                # m_d = (t_d + 1) * a_d   (d-chunk d: a at cols d*ST, t at (2+d)*ST)
                for d in range(2):
                    nc.vector.scalar_tensor_tensor(
                        m[:, d * ST : (d + 1) * ST],
                        at[:, (2 + d) * ST : (3 + d) * ST],
                        1.0,
                        at[:, d * ST : (d + 1) * ST],
                        ALU.add,
                        ALU.mult,
                    )
                ms[st] = m
            return ms

        def tail(st, m, xn):
            ap = app.tile([128, NS + LC], f32, tag="ap")
            # A columns: A[:, j] = sum_d m[d, j*128:+128]^T wc2[d]
            for j in range(NS):
                for d in range(2):
                    nc.tensor.matmul(
                        ap[:, j : j + 1],
                        m[:, d * ST + j * 128 : d * ST + (j + 1) * 128],
                        wc2[:, d : d + 1],
                        start=(d == 0),
                        stop=(d == 1),
                    )
            # u = exp(A + bc) -> ucols (bf16), also the pooling rhs
            useg = uc_sb[:, st * NS : (st + 1) * NS]
            nc.scalar.activation(useg, ap[:, 0:NS], ACT.Exp, bias=bcc[:, 0:1])
            if parts == "nopool":
                nc.vector.tensor_scalar(
                    pp_sb[:, st * LC : (st + 1) * LC],
                    ap[:, 0 : LC],
                    0.0,
                    None,
                    ALU.add,
                )
                return
            # pooling: pp[:, c] = sum_j xn[j-sub, c-chunk]^T u[:, j]
            xn3 = xn[:].rearrange("p (j l) -> p j l", j=NS)
            for c in range(LC):
                for j in range(NS):
                    nc.tensor.matmul(
                        ap[:, NS + c : NS + c + 1],
                        xn3[:, j, c * 128 : (c + 1) * 128],
                        useg[:, j : j + 1],
                        start=(j == 0),
                        stop=(j == NS - 1),
                    )
            nc.vector.tensor_scalar(
                pp_sb[:, st * LC : (st + 1) * LC],
                ap[:, NS : NS + LC],
                0.0,
                None,
                ALU.add,
            )

        groups = [
            list(range(g0, min(S, g0 + g_size))) for g0 in range(0, S, g_size)
        ]
        for _rep in range(reps):
            ats = {}
            prev = None
            for grp in groups:
                xts, xns = load_group(grp)
                ms = scores(grp, xts)
                if parts == "notails":
                    for st in grp:
                        nc.vector.tensor_scalar(
                            pp_sb[:, st * LC : (st + 1) * LC],
                            ms[st][:, 0:LC],
                            0.0,
                            None,
                            ALU.add,
                        )
                        nc.vector.tensor_scalar(
                            uc_sb[:, st * NS : (st + 1) * NS],
                            ms[st][:, 0:NS],
                            0.0,
                            None,
                            ALU.add,
                        )
                    continue
                if prev is not None:
                    for st, m, xn in prev:
                        tail(st, m, xn)
                prev = [(st, ms[st], xns[st]) for st in grp]
            if parts != "notails":
                for st, m, xn in prev:
                    tail(st, m, xn)

        nc.sync.dma_start(pp_d[:], pp_sb[:])
        nc.sync.dma_start(uc_d[:], uc_sb[:])

    _split_multiwait(nc)
    return nc


def _get_program(S, reps=1):
    key = (S, reps, G)
    if key not in _cache:
        _cache[key] = _build(S, reps)
    return _cache[key]


def _prepare(x, lengths, Wa, ba, Wb, bb, Wc, bc):
    """Host-side packing.  Returns (S, in_maps, slot_map)."""
    x = np.asarray(x, dtype=np.float32)
    lengths = np.asarray(lengths, dtype=np.int64)
    Wa = np.asarray(Wa, dtype=np.float32)
    ba = np.asarray(ba, dtype=np.float32)
    Wb = np.asarray(Wb, dtype=np.float32)
    bb = np.asarray(bb, dtype=np.float32)
    Wc = np.asarray(Wc, dtype=np.float32).reshape(D)
    bc = float(np.asarray(bc, dtype=np.float32).reshape(()))

    slots = []
    for b in range(B):
        ln = int(max(1, min(N, lengths[b])))
        for k in range((ln + ST - 1) // ST):
            slots.append((b, k * ST, min(ST, ln - k * ST)))
    n_cores = 8
    S = max(1, (len(slots) + n_cores - 1) // n_cores)

    wsb = np.zeros((128, 32, 128), dtype=FP8)
    for g, W in enumerate((Wa, Wb)):
        for dc in range(2):
            for lc in range(LC):
                wsb[:, (g * 2 + dc) * LC + lc, :] = (
                    W[lc * 128 : (lc + 1) * 128, dc * 128 : (dc + 1) * 128]
                    * W_SCALE
                ).astype(FP8)
    wsb = wsb.reshape(128, 32 * 128)
    bcol = np.stack(
        [ba[0:128], ba[128:256], bb[0:128] * 0.5, bb[128:256] * 0.5], axis=1
    ).astype(np.float32)
    wc2 = np.stack([Wc[0:128] * 0.5, Wc[128:256] * 0.5], axis=1).astype(BF16)
    bcc = np.full((128, 1), bc, dtype=np.float32)

    x8 = x.astype(FP8)
    in_maps = []
    slot_map = []
    for c in range(n_cores):
        xt8 = np.zeros((S, 128, LC, ST), dtype=FP8)
        xn8 = np.zeros((S, 128, NS, L), dtype=FP8)
        smap = []
        for k in range(S):
            gidx = c * S + k
            if gidx < len(slots):
                b, r0, nvalid = slots[gidx]
                slab = x8[b, r0 : r0 + nvalid, :]  # [nvalid, 1024]
                # xt8[k, p, lc, n] = x[b, r0+n, lc*128+p]
                xt8[k, :, :, :nvalid] = slab.reshape(nvalid, LC, 128).transpose(
                    2, 1, 0
                )
                # xn8[k, p, j, l] = x[b, r0 + j*128 + p, l]
                full = np.zeros((ST, L), dtype=FP8)
                full[:nvalid] = slab
                xn8[k] = full.reshape(NS, 128, L).transpose(1, 0, 2)
                smap.append((b, r0, nvalid))
            else:
                smap.append(None)
        in_maps.append(
            {
                "xt8": xt8.reshape(S, 128, LC * ST),
                "xn8": xn8.reshape(S, 128, NS * L),
                "wsb": wsb,
                "bcol": bcol,
                "wc2": wc2,
                "bcc": bcc,
            }
        )
        slot_map.append(smap)
    return S, in_maps, slot_map


def _assemble(results, S, slot_map):
    P = np.zeros((B, L), dtype=np.float64)
    Ssum = np.zeros(B, dtype=np.float64)
    for c in range(8):
        pp = np.asarray(results[c]["pp"], dtype=np.float64)  # [128, S*LC]
        uc = np.asarray(results[c]["uc"]).astype(np.float64)  # [128, S*NS]
        for k, slot in enumerate(slot_map[c]):
            if slot is None:
                continue
            b, r0, nvalid = slot
            cols = pp[:, k * LC : (k + 1) * LC]  # [128, LC] -> P[l = c*128+p]
            P[b] += cols.T.reshape(L)
            u = uc[:, k * NS : (k + 1) * NS]  # [128, NS]; row n = j*128 + p
            uflat = u.T.reshape(ST)
            Ssum[b] += uflat[:nvalid].sum()
    return (P / Ssum[:, None]).astype(np.float32)


def kernel(x, lengths, Wa, ba, Wb, bb, Wc, bc, _reps=1):
    from concourse.bass_utils import run_bass_kernel_spmd

    S, in_maps, slot_map = _prepare(x, lengths, Wa, ba, Wb, bb, Wc, bc)
    nc = _get_program(S, _reps)
    res = run_bass_kernel_spmd(nc, in_maps, list(range(8)))
    return _assemble(res.results, S, slot_map)
